# revision 57
# baseline (speedup 1.0000x reference)
"""Mistral GQA self-attention on 8 NeuronCores, tensor-parallel over heads.

Sharding: core c owns q-heads [4c, 4c+4) and kv-head c (q_group-aligned).
Each core computes its heads' attention output and a partial output
projection (rows 512c..512c+512 of wo); host sums the 8 fp16 partials.

v2 device scheme (fused per-token-group pipeline, per batch):
  P(tg): head-major projections (order k, v, q0..q3): per head 32
         accumulating matmuls (K=128 input-feature chunks, N=512 tokens)
         into one PSUM bank; RoPE in [d, t] layout (partition-rotated copy
         via Pool-engine copies of an fp16 SBUF staging tile; sin sign
         folded host-side); V transposed to [t, d] via DMA XBAR.
  A(tg) x W(tg-1) interleaved per head/token-block chunk:
    - S^T[k, q] = K^T_blk.T @ Q^T (PSUM), strictly-upper k-blocks skipped,
      q0 column trim on the diagonal band; causal mask added on the single
      diagonal 128x128 sub-block by an accumulating matmul with a constant
      -30000 strict-upper triangle (exp(-3e4)==0 exactly).
    - exp(S-4) -> e (fp16; the -4 cancels in 1/Z exactly).
    - Z by DVE fp16 accumulation of e tiles + one ones-matmul per (h,qg)
      (keeps the row-sum matmuls off the PE critical path).
    - PV: O^T[d, q] += V_blk.T @ e;  normalize O^T by 1/Z (DVE).
    - W(tg-1): out[t, c] accumulated over the 4 head-chunks, copied to
      fp16 staging, stored as fp16 partials (halves store traffic).
Matmul operands are fp16 (1 cyc/row); fp32 PSUM accumulation everywhere.
"""
import sys

sys.path.insert(0, "/opt/trn_rl_repo")
import numpy as np

B, T, H, D = 2, 2048, 32, 128
Q_GROUP = 4
H_KV = H // Q_GROUP
INNER = H * D          # 4096
NCORES = 8
HPC = H // NCORES      # 4 q-heads per core
ATTN_SCALE = 1.0 / np.sqrt(D)
BT = B * T             # 4096
QG = 512               # q-group (free dim of attention matmuls)
NQG = T // QG          # 4
NKB = T // 128         # 16 k-blocks
NIC = INNER // 128     # 32 contraction chunks
NTB = T // 128         # 16 token blocks per batch
NCG = INNER // 512     # 8 output column groups
NHEADS = HPC + 2       # k, v, q0..q3 per-core projection heads

_built = {}


def _split_waits(nc, mybir):
    """Walrus codegen in this container supports only 1 sync-wait per ISA
    instruction; hoist extra waits onto preceding same-engine EventSemaphore
    instructions (1 wait each)."""
    for f in nc.m.functions:
        for bb in f.blocks:
            new = []
            for inst in bb.instructions:
                si = inst.sync_info
                ow = list(si.on_wait) if si is not None and si.on_wait else []
                if len(ow) > 1:
                    for wi, w in enumerate(ow):
                        ev = mybir.InstEventSemaphore(
                            name=f"{inst.name}-wsplit{wi}",
                            ins=[], outs=[],
                            sync_info=mybir.SyncInfo(on_wait=[w], on_update=[]),
                        )
                        ev.engine = inst.engine
                        ev.debug = inst.debug
                        new.append(ev)
                    inst.sync_info = mybir.SyncInfo(
                        on_wait=[], on_update=list(si.on_update or []))
                new.append(inst)
            bb.instructions[:] = new


def _build_causal():
    import concourse.bass as bass
    import concourse.mybir as mybir
    import concourse.tile as tile
    from contextlib import ExitStack

    F32 = mybir.dt.float32
    FR = mybir.dt.float16
    EXP = mybir.ActivationFunctionType.Exp

    nc = bass.Bass(trn_type="TRN2", target_bir_lowering=False, debug=False)
    # host-repacked inputs (see _run): partition-major layouts so every DMA
    # moves >=1KB contiguous runs.
    xTr = nc.dram_tensor("xTr", [128, NIC, BT], FR, kind="ExternalInput").ap()
    # head order: k, v, q0..q3; per head a [128, NIC*128] slab
    wqkv = nc.dram_tensor("wqkv", [128, NHEADS, NIC * 128], FR,
                          kind="ExternalInput").ap()
    woh = nc.dram_tensor("woh", [128, HPC, INNER], FR,
                         kind="ExternalInput").ap()
    cosT = nc.dram_tensor("cosT", [D, T], FR, kind="ExternalInput").ap()
    sinTs = nc.dram_tensor("sinTs", [D, T], FR, kind="ExternalInput").ap()
    triM = nc.dram_tensor("triM", [128, 128], FR, kind="ExternalInput").ap()
    out = nc.dram_tensor("out", [BT, INNER], FR, kind="ExternalOutput").ap()

    with tile.TileContext(nc) as tc, ExitStack() as ctx:
        const = ctx.enter_context(tc.tile_pool(name="const", bufs=1))
        sb = ctx.enter_context(tc.tile_pool(name="sb", bufs=1))
        ps = ctx.enter_context(tc.tile_pool(name="ps", bufs=1, space="PSUM"))

        # ---- weight / const loads (once; order matters for startup) ----
        w_sb = const.tile([128, NHEADS, NIC * 128], FR)

        x_tiles = {}

        def issue_x(b, tg, winterleave=False):
            grp = []
            for g in range(8):
                if winterleave:  # pair k/v-weight chunks with the x stream
                    for i in range(2):
                        nc.sync.dma_start(
                            out=w_sb[:, i, g * 512:(g + 1) * 512],
                            in_=wqkv[:, i, g * 512:(g + 1) * 512])
                xt = sb.tile([128, 4 * QG], FR, tag="x", bufs=12,
                             name=f"x_{b}_{tg}_{g}")
                tc0 = b * T + tg * QG
                nc.sync.dma_start(
                    out=xt, in_=xTr[:, g * 4:(g + 1) * 4, tc0:tc0 + QG])
                grp.append(xt)
            x_tiles[(b, tg)] = grp

        issue_x(0, 0, winterleave=True)
        # first q head's weights next (its pass starts right after k/v);
        # remaining bulk loads are emitted interleaved into P(0,0)'s
        # epilogues (bulk_q below) so they queue behind the latency-critical
        # rot/transpose DMAs yet land just before their consumers
        HW = NIC * 64  # half a head slab
        nc.sync.dma_start(out=w_sb[:, 2, :HW], in_=wqkv[:, 2, :HW])
        nc.sync.dma_start(out=w_sb[:, 2, HW:], in_=wqkv[:, 2, HW:])
        cos_sb = const.tile([D, T], FR)
        sin_sb = const.tile([D, T], FR)
        nc.sync.dma_start(out=cos_sb, in_=cosT)
        nc.sync.dma_start(out=sin_sb, in_=sinTs)
        trim_sb = const.tile([128, 128], FR)
        nc.sync.dma_start(out=trim_sb, in_=triM)
        ones16 = const.tile([128, 128], FR)
        nc.gpsimd.memset(ones16, 1.0)
        biasm4 = const.tile([128, 1], F32)
        nc.gpsimd.memset(biasm4, -4.0)
        wo_sb = const.tile([128, HPC, INNER], FR)

        def _wload(i):
            QW = NIC * 32  # quarter slab: finer arrival for the pass start
            for q in range(4):
                nc.sync.dma_start(out=w_sb[:, i, q * QW:(q + 1) * QW],
                                  in_=wqkv[:, i, q * QW:(q + 1) * QW])

        def _woload(h):
            nc.sync.dma_start(out=wo_sb[:, h, :], in_=woh[:, h, :])

        bulk_q = [lambda: _wload(3), lambda: _wload(4), lambda: _wload(5),
                  lambda: (_woload(0), _woload(1)),
                  lambda: (_woload(2), _woload(3))]

        steps = [(b, tg) for b in range(B) for tg in range(NQG)]

        def emit_P_epilogue(b, tg, i, prj, qt):
            kt = kv_tiles[b % 2]["kt"]
            v_sb = kv_tiles[b % 2]["v"]
            if i == 1:  # v: transpose [d, t] -> [t, d] blocks via XBAR
                vtmp = sb.tile([128, QG], FR, tag="vtmp", bufs=2,
                               name=f"vtmp_{b}_{tg}")
                nc.scalar.copy(vtmp, prj)
                for j in range(QG // 128):
                    nc.sync.dma_start(
                        out=v_sb[:, tg * 4 + j, :],
                        in_=vtmp[:, j * 128:(j + 1) * 128],
                        transpose=True)
            else:  # k or q: RoPE
                p_sb = sb.tile([128, QG], FR, tag="psb", bufs=2,
                               name=f"psb_{b}_{tg}_{i}")
                nc.scalar.copy(p_sb, prj)
                rot = sb.tile([128, QG], FR, tag="rot", bufs=2,
                              name=f"rot_{b}_{tg}_{i}")
                nc.sync.dma_start(out=rot[0:64, :], in_=p_sb[64:128, :])
                nc.sync.dma_start(out=rot[64:128, :], in_=p_sb[0:64, :])
                cs = cos_sb[:, tg * QG:(tg + 1) * QG]
                ss = sin_sb[:, tg * QG:(tg + 1) * QG]
                a_t = sb.tile([128, QG], FR, tag="ropea", bufs=2,
                              name=f"ra_{b}_{tg}_{i}")
                nc.vector.tensor_mul(a_t, p_sb, cs)
                b_t = sb.tile([128, QG], FR, tag="ropeb", bufs=2,
                              name=f"rb_{b}_{tg}_{i}")
                nc.vector.tensor_mul(b_t, rot, ss)
                if i == 0:  # k
                    nc.vector.tensor_add(
                        kt[:, tg * QG:(tg + 1) * QG], a_t, b_t)
                else:       # q head i-2
                    qth = sb.tile([128, QG], FR, tag=f"qt{i-2}", bufs=2,
                                  name=f"qt_{b}_{tg}_{i-2}")
                    nc.vector.tensor_add(qth, a_t, b_t)
                    qt.append(qth)

        def emit_P(b, tg):
            """Projections for token group tg of batch b (head-major)."""
            xg = x_tiles[(b, tg)]
            qt = []
            if (b, tg) == (0, 0):
                # k/v passes paired at ic-quad granularity: the startup x/w
                # DMA stream feeds 8 matmuls per group instead of 4
                prjs = [ps.tile([128, QG], F32, tag="pw", bufs=3,
                                name=f"prj_0_0_{i}") for i in range(2)]
                for g in range(8):
                    for i in range(2):
                        for ic in range(4 * g, 4 * g + 4):
                            nc.tensor.matmul(
                                prjs[i],
                                w_sb[:, i, ic * 128:(ic + 1) * 128],
                                xg[g][:, (ic % 4) * QG:(ic % 4 + 1) * QG],
                                start=(ic == 0), stop=(ic == NIC - 1))
                for i in range(2):
                    emit_P_epilogue(b, tg, i, prjs[i], qt)
                    if bulk_q:
                        bulk_q.pop(0)()
                first = 2
            else:
                first = 0
            for i in range(first, NHEADS):
                emit_P_head(b, tg, i, qt)
            return qt

        def emit_P_head(b, tg, i, qt):
            xg = x_tiles[(b, tg)]
            prj = ps.tile([128, QG], F32, tag="pw", bufs=3,
                          name=f"prj_{b}_{tg}_{i}")
            for ic in range(NIC):
                nc.tensor.matmul(
                    prj,
                    w_sb[:, i, ic * 128:(ic + 1) * 128],
                    xg[ic // 4][:, (ic % 4) * QG:(ic % 4 + 1) * QG],
                    start=(ic == 0), stop=(ic == NIC - 1))
            emit_P_epilogue(b, tg, i, prj, qt)
            if bulk_q:
                bulk_q.pop(0)()

        def emit_A_head(b, tg, h, qt):
            """Attention S/exp/accum/PV chain for one head at qg=tg.
            Returns (o_ps, acc) to finish later (z/recip/norm)."""
            kt = kv_tiles[b % 2]["kt"]
            v_sb = kv_tiles[b % 2]["v"]
            kmax = 4 * tg + 4
            o_ps = ps.tile([D, QG], F32, tag="o", bufs=2,
                           name=f"o_{b}_{tg}_{h}")
            acc = sb.tile([128, QG], FR, tag="acc", bufs=3,
                          name=f"acc_{b}_{tg}_{h}")
            for kb in range(kmax):
                q0 = max(0, 128 * (kb - 4 * tg))
                s_ps = ps.tile([128, QG], F32, tag="sz", bufs=3,
                               name=f"s_{b}_{tg}_{h}_{kb}")
                diag = kb >= 4 * tg
                nc.tensor.matmul(
                    s_ps[:, q0:], kt[:, kb * 128:(kb + 1) * 128],
                    qt[h][:, q0:], start=True, stop=True)
                e_sb = sb.tile([128, QG], FR, tag="e", bufs=8,
                               name=f"e_{b}_{tg}_{h}_{kb}")
                nc.scalar.activation(e_sb[:, q0:], s_ps[:, q0:], EXP,
                                     bias=biasm4)
                if diag:
                    # causal mask: zero the strictly-lower (k > q') entries
                    # of the diagonal 128x128 sub-block on the idle Pool
                    # engine instead of -30000 matmul-adds on the PE
                    nc.vector.tensor_mul(e_sb[:, q0:q0 + 128],
                                         e_sb[:, q0:q0 + 128], trim_sb)
                if kb == 0:
                    nc.vector.tensor_copy(acc, e_sb)
                else:
                    nc.vector.tensor_add(acc[:, q0:], acc[:, q0:],
                                         e_sb[:, q0:])
                nc.tensor.matmul(
                    o_ps[:, q0:], v_sb[:, kb, :], e_sb[:, q0:],
                    start=(kb == 0), stop=(kb == kmax - 1))
            return o_ps, acc

        def emit_A_finish(b, tg, h, o_ps, acc):
            z_ps = ps.tile([128, QG], F32, tag="sz", bufs=3,
                           name=f"z_{b}_{tg}_{h}")
            nc.tensor.matmul(z_ps, ones16, acc, start=True, stop=True)
            r_sb = sb.tile([128, QG], F32, tag="r", bufs=2,
                           name=f"r_{b}_{tg}_{h}")
            nc.vector.reciprocal(r_sb, z_ps)
            oth = sb.tile([D, QG], FR, tag=f"ot{h}", bufs=2,
                          name=f"ot_{b}_{tg}_{h}")
            nc.vector.tensor_mul(oth, o_ps, r_sb)
            return oth

        def emit_W_tb(pb, ptg, tbl, ot, final=False):
            """Output projection for one 128-token block of a previous tg."""
            t0 = pb * T + ptg * QG + tbl * 128
            for cgp in range(4):
                o_sb = sb.tile([128, 1024], FR, tag="osb", bufs=4,
                               name=f"osb_{pb}_{ptg}_{tbl}_{cgp}")
                for half in range(2):
                    cg = cgp * 2 + half
                    op = ps.tile([128, 512], F32, tag="pw", bufs=3,
                                 name=f"op_{pb}_{ptg}_{tbl}_{cg}")
                    for h in range(HPC):
                        nc.tensor.matmul(
                            op, ot[h][:, tbl * 128:(tbl + 1) * 128],
                            wo_sb[:, h, cg * 512:(cg + 1) * 512],
                            start=(h == 0), stop=(h == HPC - 1))
                    nc.any.tensor_copy(
                        o_sb[:, half * 512:(half + 1) * 512], op)
                    if final:  # store each half as soon as it's staged
                        nc.sync.dma_start(
                            out=out[t0:t0 + 128, cg * 512:(cg + 1) * 512],
                            in_=o_sb[:, half * 512:(half + 1) * 512])
                if not final:
                    nc.sync.dma_start(
                        out=out[t0:t0 + 128, cgp * 1024:(cgp + 1) * 1024],
                        in_=o_sb)

        kv_tiles = [None, None]  # per-batch kt / v tiles (bufs=2 rotation)

        prev = None      # (b, tg, ot_tiles) awaiting output projection
        qt_next = None   # P(0,1) result when pre-emitted inside step 0
        for si, (b, tg) in enumerate(steps):
            if tg == 0:
                kv_tiles[b % 2] = {
                    "kt": sb.tile([D, T], FR, tag="kt", bufs=2,
                                  name=f"kt_{b}"),
                    "v": sb.tile([128, NKB, D], FR, tag="v", bufs=2,
                                 name=f"v_{b}"),
                }
            qt = qt_next if qt_next is not None else emit_P(b, tg)
            qt_next = None
            if si + 1 < len(steps):
                issue_x(*steps[si + 1])
            ot = []
            if prev is None:
                # first segment: no W work to hide exp latency — interleave
                # the next token group's projection passes instead
                qt_next = []
                pend = None
                for h in range(HPC):
                    o_ps, acc = emit_A_head(b, tg, h, qt)
                    if pend is not None:
                        ot.append(emit_A_finish(b, tg, pend[0], *pend[1:]))
                    if h >= 1:
                        emit_P_head(*steps[si + 1], h - 1, qt_next)
                    pend = (h, o_ps, acc)
                ot.append(emit_A_finish(b, tg, pend[0], *pend[1:]))
                for i in range(3, NHEADS):
                    emit_P_head(*steps[si + 1], i, qt_next)
            else:
                # A(tg) interleaved with W(prev)
                pend = None
                for h in range(HPC):
                    o_ps, acc = emit_A_head(b, tg, h, qt)
                    if pend is not None:
                        ot.append(emit_A_finish(b, tg, pend[0], *pend[1:]))
                    emit_W_tb(prev[0], prev[1], h, prev[2])
                    pend = (h, o_ps, acc)
                ot.append(emit_A_finish(b, tg, pend[0], *pend[1:]))
            prev = (b, tg, ot)
        for tbl in range(4):
            emit_W_tb(prev[0], prev[1], tbl, prev[2], final=True)
    _split_waits(nc, mybir)
    return nc


def _build_generic():
    """Fallback for non-canonical masks: straightforward per-batch scheme
    with explicit mask_w/mask_b application (from the v1 kernel)."""
    import concourse.bass as bass
    import concourse.mybir as mybir
    import concourse.tile as tile
    from concourse.masks import make_identity
    from contextlib import ExitStack

    F32 = mybir.dt.float32
    FR = mybir.dt.float16
    EXP = mybir.ActivationFunctionType.Exp

    nc = bass.Bass(trn_type="TRN2", target_bir_lowering=False, debug=False)
    xT = nc.dram_tensor("xT", [INNER, BT], FR, kind="ExternalInput").ap()
    wq = nc.dram_tensor("wq", [INNER, HPC * D], FR, kind="ExternalInput").ap()
    wkv = nc.dram_tensor("wkv", [INNER, 2 * D], FR, kind="ExternalInput").ap()
    wo = nc.dram_tensor("wo", [HPC * D, INNER], FR, kind="ExternalInput").ap()
    cosT = nc.dram_tensor("cosT", [D, T], F32, kind="ExternalInput").ap()
    sinTs = nc.dram_tensor("sinTs", [D, T], F32, kind="ExternalInput").ap()
    mwTf = nc.dram_tensor("mwTf", [T, T], F32, kind="ExternalInput").ap()
    mbTf = nc.dram_tensor("mbTf", [T, T], F32, kind="ExternalInput").ap()
    out = nc.dram_tensor("out", [BT, INNER], F32, kind="ExternalOutput").ap()

    with tile.TileContext(nc) as tc, ExitStack() as ctx:
        const = ctx.enter_context(tc.tile_pool(name="const", bufs=1))
        cos_sb = const.tile([D, T], F32)
        sin_sb = const.tile([D, T], F32)
        nc.sync.dma_start(out=cos_sb, in_=cosT)
        nc.sync.dma_start(out=sin_sb, in_=sinTs)
        ones_col = const.tile([128, 128], FR)
        nc.gpsimd.memset(ones_col, 1.0)
        ident = const.tile([128, 128], F32)
        make_identity(nc, ident)
        biasm4 = const.tile([128, 1], F32)
        nc.gpsimd.memset(biasm4, -4.0)

        for b in range(B):
            t0 = b * T
            with tc.tile_pool(name=f"bp{b}", bufs=1) as bp:
                qt_sb = [bp.tile([D, T], FR, tag=f"qt{h}", name=f"qt{h}")
                         for h in range(HPC)]
                kt_sb = bp.tile([D, T], FR, tag="kt")
                v_sb = bp.tile([128, NKB, D], FR, tag="v")
                with tc.tile_pool(name="wproj", bufs=1) as wpool, \
                     tc.tile_pool(name="xin", bufs=8) as xpool, \
                     tc.tile_pool(name="peps", bufs=3) as epool, \
                     tc.tile_pool(name="pps", bufs=1, space="PSUM") as pps, \
                     tc.tile_pool(name="tps", bufs=2, space="PSUM") as tps:
                    w_sb = wpool.tile([128, NIC, (HPC + 2) * D], FR)
                    for ic in range(NIC):
                        nc.sync.dma_start(
                            out=w_sb[:, ic, : HPC * D],
                            in_=wq[ic * 128:(ic + 1) * 128, :])
                        nc.sync.dma_start(
                            out=w_sb[:, ic, HPC * D:],
                            in_=wkv[ic * 128:(ic + 1) * 128, :])
                    for tg in range(NQG):
                        tc0 = t0 + tg * QG
                        prj = [pps.tile([128, QG], F32, tag=f"prj{i}",
                                        name=f"prj{i}")
                               for i in range(HPC + 2)]
                        for ic in range(NIC):
                            x_sb = xpool.tile([128, QG], FR)
                            nc.sync.dma_start(
                                out=x_sb,
                                in_=xT[ic * 128:(ic + 1) * 128, tc0:tc0 + QG])
                            for i in range(HPC + 2):
                                nc.tensor.matmul(
                                    prj[i],
                                    w_sb[:, ic, i * D:(i + 1) * D],
                                    x_sb,
                                    start=(ic == 0), stop=(ic == NIC - 1))
                        cs = cos_sb[:, tg * QG:(tg + 1) * QG]
                        ss = sin_sb[:, tg * QG:(tg + 1) * QG]
                        for i in range(HPC + 1):
                            psr = prj[i]
                            p_sb = epool.tile([128, QG], F32, tag="psb")
                            nc.scalar.copy(p_sb, psr)
                            rot = epool.tile([128, QG], F32, tag="rot")
                            nc.sync.dma_start(out=rot[0:64, :],
                                              in_=p_sb[64:128, :])
                            nc.sync.dma_start(out=rot[64:128, :],
                                              in_=p_sb[0:64, :])
                            a_t = epool.tile([128, QG], F32, tag="ropea")
                            nc.vector.tensor_mul(a_t, p_sb, cs)
                            b_t = epool.tile([128, QG], F32, tag="ropeb")
                            nc.vector.tensor_mul(b_t, rot, ss)
                            dst = qt_sb[i] if i < HPC else kt_sb
                            nc.vector.tensor_add(
                                dst[:, tg * QG:(tg + 1) * QG], a_t, b_t)
                        vtmp = epool.tile([128, QG], F32, tag="vtmp")
                        nc.scalar.copy(vtmp, prj[HPC + 1])
                        for j in range(QG // 128):
                            vt_ps = tps.tile([128, 128], F32, tag="vt")
                            nc.tensor.transpose(
                                vt_ps, vtmp[:, j * 128:(j + 1) * 128], ident)
                            nc.vector.tensor_copy(v_sb[:, tg * 4 + j, :],
                                                  vt_ps)

                ot_sb = [bp.tile([D, T], FR, tag=f"ot{h}", name=f"ot{h}")
                         for h in range(HPC)]
                with tc.tile_pool(name="exps", bufs=24) as spool, \
                     tc.tile_pool(name="asml", bufs=4) as apool, \
                     tc.tile_pool(name="sps", bufs=5, space="PSUM") as sps, \
                     tc.tile_pool(name="ops", bufs=2, space="PSUM") as ops, \
                     tc.tile_pool(name="zps", bufs=1, space="PSUM") as zps:
                    for h in range(HPC):
                        for qg in range(NQG):
                            qs = qt_sb[h][:, qg * QG:(qg + 1) * QG]
                            kmax = NKB
                            o_ps = ops.tile([D, QG], F32, tag="o")
                            z_ps = zps.tile([128, QG], F32, tag="z")
                            for kb in range(kmax):
                                s_ps = sps.tile([128, QG], F32, tag="s")
                                nc.tensor.matmul(
                                    s_ps,
                                    kt_sb[:, kb * 128:(kb + 1) * 128],
                                    qs, start=True, stop=True)
                                mw_t = apool.tile([128, QG], F32, tag="mw")
                                nc.sync.dma_start(
                                    out=mw_t,
                                    in_=mwTf[kb * 128:(kb + 1) * 128,
                                             qg * QG:(qg + 1) * QG])
                                mb_t = apool.tile([128, QG], F32, tag="mb")
                                nc.sync.dma_start(
                                    out=mb_t,
                                    in_=mbTf[kb * 128:(kb + 1) * 128,
                                             qg * QG:(qg + 1) * QG])
                                nc.vector.tensor_mul(s_ps, s_ps, mw_t)
                                nc.vector.tensor_add(s_ps, s_ps, mb_t)
                                e_sb = spool.tile([128, QG], FR, tag="e")
                                nc.scalar.activation(e_sb, s_ps, EXP,
                                                     bias=biasm4)
                                nc.tensor.matmul(
                                    z_ps, ones_col, e_sb,
                                    start=(kb == 0), stop=(kb == kmax - 1))
                                nc.tensor.matmul(
                                    o_ps, v_sb[:, kb, :], e_sb,
                                    start=(kb == 0), stop=(kb == kmax - 1))
                            r_sb = apool.tile([128, QG], F32, tag="r")
                            nc.vector.reciprocal(r_sb, z_ps)
                            nc.vector.tensor_mul(
                                ot_sb[h][:, qg * QG:(qg + 1) * QG],
                                o_ps, r_sb)

                with tc.tile_pool(name="wom", bufs=2) as wopool, \
                     tc.tile_pool(name="wos", bufs=6) as wosb, \
                     tc.tile_pool(name="wops", bufs=4, space="PSUM") as wps:
                    for cg in range(NCG):
                        wo_sb = wopool.tile([128, HPC, 512], FR, tag="wo")
                        for h in range(HPC):
                            nc.sync.dma_start(
                                out=wo_sb[:, h, :],
                                in_=wo[h * 128:(h + 1) * 128,
                                       cg * 512:(cg + 1) * 512])
                        for tb in range(NTB):
                            op = wps.tile([128, 512], F32, tag="op")
                            for h in range(HPC):
                                nc.tensor.matmul(
                                    op,
                                    ot_sb[h][:, tb * 128:(tb + 1) * 128],
                                    wo_sb[:, h, :],
                                    start=(h == 0), stop=(h == HPC - 1))
                            o_sb = wosb.tile([128, 512], F32, tag="osb")
                            nc.any.tensor_copy(o_sb, op)
                            nc.sync.dma_start(
                                out=out[t0 + tb * 128:t0 + (tb + 1) * 128,
                                        cg * 512:(cg + 1) * 512],
                                in_=o_sb)
    _split_waits(nc, mybir)
    return nc


def _get(variant):
    if variant not in _built:
        _built[variant] = (_build_causal() if variant == "causal"
                           else _build_generic())
    return _built[variant]


def _canonical_causal(mask_w, mask_b):
    tri = np.tril(np.ones((T, T), dtype=np.float32))
    if not np.array_equal(mask_w, tri):
        return False
    off = mask_b[tri == 0]
    if off.size and not (np.all(off <= -20000.0) and np.ptp(off) == 0):
        return False
    return bool(np.all(mask_b[tri == 1] == 0.0))


def _run(stm, wq, wk, wv, wo, cos, sin, mask_w, mask_b, trace=False):
    from concourse.bass_utils import run_bass_kernel_spmd

    BF = np.float16
    x = np.ascontiguousarray(np.asarray(stm).reshape(BT, INNER))
    wq = np.asarray(wq); wk = np.asarray(wk); wv = np.asarray(wv)
    wo = np.asarray(wo)
    cosT = np.ascontiguousarray(np.asarray(cos).T)
    sinT = np.ascontiguousarray(np.asarray(sin).T)
    sinTs = sinT.copy()
    sinTs[: D // 2] *= -1.0
    mask_w = np.asarray(mask_w); mask_b = np.asarray(mask_b)
    causal = _canonical_causal(mask_w, mask_b)
    variant = "causal" if causal else "generic"
    nc = _get(variant)
    wq_s = (wq * ATTN_SCALE)

    in_maps = []
    if causal:
        # x^T rows permuted to [partition, chunk, token]
        xTr = np.ascontiguousarray(
            x.T.reshape(NIC, 128, BT).transpose(1, 0, 2)).astype(BF)
        triM = np.triu(np.ones((128, 128), dtype=np.float32))
        for c in range(NCORES):
            wcat = np.concatenate(
                [wk[:, c * D:(c + 1) * D], wv[:, c * D:(c + 1) * D],
                 wq_s[:, c * HPC * D:(c + 1) * HPC * D]], axis=1)
            # [INNER, 6*128] -> [128, 6, NIC*128] (head-major slabs,
            # ic-contiguous per head)
            wqkv = wcat.reshape(NIC, 128, NHEADS, 128).transpose(
                1, 2, 0, 3).reshape(128, NHEADS, NIC * 128)
            woc = wo[c * HPC * D:(c + 1) * HPC * D, :].reshape(
                HPC, 128, INNER).transpose(1, 0, 2)
            m = {
                "xTr": xTr,
                "wqkv": np.ascontiguousarray(wqkv).astype(BF),
                "woh": np.ascontiguousarray(woc).astype(BF),
                "cosT": cosT.astype(BF),
                "sinTs": sinTs.astype(BF),
                "triM": triM.astype(BF),
            }
            in_maps.append(m)
    else:
        xT = np.ascontiguousarray(x.T).astype(BF)
        for c in range(NCORES):
            m = {
                "xT": xT,
                "wq": np.ascontiguousarray(
                    wq_s[:, c * HPC * D:(c + 1) * HPC * D]).astype(BF),
                "wkv": np.ascontiguousarray(
                    np.concatenate([wk[:, c * D:(c + 1) * D],
                                    wv[:, c * D:(c + 1) * D]],
                                   axis=1)).astype(BF),
                "wo": np.ascontiguousarray(
                    wo[c * HPC * D:(c + 1) * HPC * D, :]).astype(BF),
                "cosT": cosT,
                "sinTs": sinTs,
                "mwTf": np.ascontiguousarray(mask_w.T),
                "mbTf": np.ascontiguousarray(mask_b.T),
            }
            in_maps.append(m)

    res = run_bass_kernel_spmd(nc, in_maps, core_ids=list(range(NCORES)),
                               trace=trace)
    acc = res.results[0]["out"].astype(np.float64)
    for c in range(1, NCORES):
        acc += res.results[c]["out"]
    full = acc.astype(np.float32).reshape(B, T, H, D)
    return full, res


def kernel(stm, wq, wk, wv, wo, cos, sin, mask_w, mask_b):
    out, _ = _run(stm, wq, wk, wv, wo, cos, sin, mask_w, mask_b, trace=False)
    return out
